# revision 30
# baseline (speedup 1.0000x reference)
"""Grouped GEMM (MoE routing) Trainium2 kernel.

Strategy: tensor-parallel shard of the output N dim across 8 NeuronCores.
Every core sees all T=8192 tokens and a 512-wide slice of every expert's
weights, so per-core work is identical regardless of segment sizes and a
single SPMD program (with the segment boundaries baked in as compile-time
constants) runs on all 8 cores.

Inputs are cast to bf16 on the host (rel-err ~3e-3, well inside the 2e-2
gate), halving per-core HBM read traffic so the DMA hides under the matmul
roofline (~437 us/core).  The output is written back as bf16 too and
upcast on the host.

Token blocks (<=512 tokens, segment-aligned) are the unit of work.  The
host lays `a` out as one CONTIGUOUS slab per block ([P, KO, Lc]) and the
output as one slab per block ([P, NB*L]), so every DMA descriptor run is
multi-KB: the SDMA round-robin between the a-queue and the weight-queue is
packet-granular, and sub-KB a-descriptors would get starved ~9:1 whenever
a 4MB weight tile streams in.

Per core:  out_t[n, t] = sum_k w_t[e(t), k, n] * a_t[k, t]
  - a{b} : [P, KO, Lc_b]  per-block a slab, k-partition-major (shared)
  - wt   : [EA, P, KO, NS] per-core weight slices, k-partition-major
  - o{b} : [P, NB*L_b] bf16 per-block output slab

Matmul mapping: stationary lhsT = w tile [k=128, n=128] bf16, moving rhs =
a tile [k=128, tok<=512] bf16, PSUM out [n=128, tok<=512] fp32, accumulated
over the 32 k-chunks.

DMA ring split: a-tiles ride the sync (SP) HWDGE ring; weight loads and
output writes ride the scalar (ACT) ring.  Each expert's weight tile is
loaded in 1MB ko-chunks spread token-weighted over the blocks of the two
PREVIOUS expert runs (the wpool buffer is provably free by then), so there
is no weight burst competing with a-tiles at an expert switch.  Block 0
uses graded ko batches (2,2,4,8,16) so the first matmul starts ~10us in;
the last block is shrunk to 128 tokens so the final drain is short.
"""

import numpy as np
import ml_dtypes

import concourse.bacc as bacc
import concourse.bass as bass
import concourse.mybir as mybir
import concourse.tile as tile
from concourse.bass_utils import run_bass_kernel_spmd

NC = 8          # NeuronCores
P = 128         # partitions
TB = 512        # max token block (moving free dim, one PSUM bank of fp32)
KOC = 32        # k-chunks per a-tile DMA batch

BF16 = ml_dtypes.bfloat16

LAST_RESULT = {}


def _token_blocks(seg_starts, seg_ends):
    """Split each segment into even pieces of <=512 tokens; carve a small
    tail piece off the last segment so the final drain is short."""
    blocks = []  # (tstart, tlen, run_idx)
    nseg = len(seg_starts)
    for widx, (s, t) in enumerate(zip(seg_starts, seg_ends)):
        ln = t - s
        tail = 128 if (widx == nseg - 1 and ln > 256) else 0
        mid = ln - tail
        pieces = []
        if mid > 0:
            npieces = max(1, -(-mid // TB))
            base, rem = divmod(mid, npieces)
            pieces += [base + (1 if i < rem else 0) for i in range(npieces)]
        if tail:
            pieces.append(tail)
        p = s
        for L in pieces:
            if L > 0:
                blocks.append((p, L, widx))
                p += L
    return blocks


def _w_chunks(run_idx, KO):
    """ko-chunk splits for one expert's weight tile: (start_ko, n_ko)."""
    if run_idx == 0 and KO == 32:
        return [(0, 1), (1, 1), (2, 2), (4, 4), (8, 8), (16, 8), (24, 8)]
    step = 8 if KO % 8 == 0 else KO
    return [(ko, min(step, KO - ko)) for ko in range(0, KO, step)]


def _kon_sched(bi, KO, koc_n):
    """ko batch sizes for one block's a-tile DMAs; graded for block 0 so
    the first matmul only waits on a small first transfer."""
    if bi == 0 and KO >= 16 and (KO - 16) % koc_n == 0:
        return [1, 1, 2, 4, 8] + [koc_n] * ((KO - 16) // koc_n)
    return [koc_n] * (KO // koc_n)


def _build_program(T, K, NS, EA, blocks):
    f32 = mybir.dt.float32
    bf16 = mybir.dt.bfloat16
    KO = K // P
    NB = NS // P
    koc_n = min(KOC, KO)

    nc = bacc.Bacc(None, target_bir_lowering=False)
    wt = nc.declare_dram_parameter("wt", [EA, P, KO, NS], bf16, isOutput=False)
    a_prm = []
    o_prm = []
    for bi, (ts, L, widx) in enumerate(blocks):
        Lc = L + (L % 2)
        a_prm.append(nc.declare_dram_parameter(
            f"a{bi}", [P, KO, Lc], bf16, isOutput=False))
        o_prm.append(nc.declare_dram_parameter(
            f"o{bi}", [P, NB * L], bf16, isOutput=True))

    run_of_block = [b[2] for b in blocks]
    nrun = EA

    # prefetch plan: run r's weight chunks are spread token-weighted across
    # the blocks of runs r-2 and r-1 (run r's wpool buffer was last read by
    # run r-3, so it is free throughout), avoiding any weight burst at an
    # expert switch.  Run 0's chunks are emitted before the first block.
    blocks_of_run = {}
    for bi, r in enumerate(run_of_block):
        blocks_of_run.setdefault(r, []).append(bi)
    emit_after = {bi: [] for bi in range(len(blocks))}   # bi -> [(run, chunk)]
    for r in range(1, nrun):
        hosts = (blocks_of_run[r - 2] if r >= 2 else []) + blocks_of_run[r - 1]
        host_tok = [blocks[bi][1] for bi in hosts]
        cum = np.cumsum(host_tok)
        total = cum[-1]
        chunks = _w_chunks(r, KO)
        for ci, ch in enumerate(chunks):
            frac = (ci + 0.5) / len(chunks)
            if r == 1:
                # run 1's chunks compete with the first blocks' a-tiles
                # inside the slow DMA spin-up window; push them into the
                # second half of run 0's token span.
                frac = 0.5 + 0.5 * frac
            pos = frac * total
            host_bi = hosts[int(np.searchsorted(cum, pos))]
            emit_after[host_bi].append((r, ch))

    with tile.TileContext(nc) as tc:
        with (
            tc.tile_pool(name="wpool", bufs=3) as wpool,
            tc.tile_pool(name="apool", bufs=3) as apool,
            tc.tile_pool(name="opool", bufs=3) as opool,
            tc.tile_pool(name="warm", bufs=1) as warmpool,
            tc.tile_pool(name="psum", bufs=8, space=bass.MemorySpace.PSUM) as psum_pool,
        ):
            wtiles = {}

            def emit_w_chunk(r, ch):
                if r not in wtiles:
                    wtiles[r] = wpool.tile([P, KO, NS], bf16, tag="w",
                                           name=f"w{r % 3}")
                ko0, kon = ch
                nc.scalar.dma_start(
                    out=wtiles[r][:, ko0:ko0 + kon, :],
                    in_=wt[r, :, ko0:ko0 + kon, :],
                )

            # PE warmup: a few dummy matmuls on a zeroed scratch tile while
            # the first real DMAs stream in, nudging the HAM clock gate
            # toward 8/8 before real matmuls start.
            scr = warmpool.tile([P, 256], bf16, tag="scr", name="warm_scr")
            nc.vector.memset(scr[:, :], 0)
            warm_ps = psum_pool.tile([P, 128], f32, tag="ps", name="warm_ps",
                                     padded_shape=[P, TB])
            for _ in range(10):
                nc.tensor.matmul(warm_ps[:, :], scr[:, 0:128], scr[:, 128:256],
                                 start=True, stop=True)

            for ch in _w_chunks(0, KO):
                emit_w_chunk(0, ch)

            for bi, (ts, L, widx) in enumerate(blocks):
                r = run_of_block[bi]
                w_tile = wtiles[r]
                Lc = L + (L % 2)
                ptiles = [psum_pool.tile([P, Lc], f32, tag="ps", name=f"ps{nb}",
                                         padded_shape=[P, TB])
                          for nb in range(NB)]
                ko0 = 0
                for kon in _kon_sched(bi, KO, koc_n):
                    # flat [P, kon*Lc] so src AND dst are fully contiguous
                    a_tile = apool.tile([P, kon * Lc], bf16, tag="a",
                                        name="a_tile",
                                        padded_shape=[P, koc_n * TB])
                    nc.sync.dma_start(
                        out=a_tile[:, :],
                        in_=a_prm[bi][:, ko0:ko0 + kon, :],
                    )
                    for koi in range(kon):
                        ko = ko0 + koi
                        for nb in range(NB):
                            nc.tensor.matmul(
                                ptiles[nb][:, :],
                                w_tile[:, ko, nb * P:(nb + 1) * P],
                                a_tile[:, koi * Lc:(koi + 1) * Lc],
                                start=(ko == 0),
                                stop=(ko == KO - 1),
                            )
                    ko0 += kon
                o_tile = opool.tile([P, NB * L], bf16, tag="o", name="o_tile",
                                    padded_shape=[P, NB * TB])
                for nb in range(NB):
                    nc.vector.tensor_copy(o_tile[:, nb * L:(nb + 1) * L],
                                          ptiles[nb][:, 0:L])
                nc.scalar.dma_start(out=o_prm[bi][:, :], in_=o_tile[:, :])
                for (rr, ch) in emit_after[bi]:
                    emit_w_chunk(rr, ch)
    nc.compile()
    return nc


def kernel(a, b, c, seg_indptr, weight_indices, batch_size, **_):
    T, K = a.shape
    E, N, K2 = b.shape
    assert K == K2
    NS = N // NC

    seg = np.asarray(seg_indptr).astype(np.int64)
    widx_arr = np.asarray(weight_indices).astype(np.int64)
    segs = [(int(seg[e]), int(seg[e + 1]), int(widx_arr[e]))
            for e in range(int(batch_size)) if seg[e + 1] > seg[e]]
    seg_starts = [s for s, _, _ in segs]
    seg_ends = [t for _, t, _ in segs]
    experts = [w for _, _, w in segs]
    EA = len(segs)
    blocks = _token_blocks(seg_starts, seg_ends)

    KO = K // P
    NB = NS // P
    a16 = np.asarray(a, dtype=np.float32).astype(BF16)        # [T, K]

    # per-block contiguous a slabs: a{b}[p, ko, t'] = a[ts + t', ko*128 + p]
    a_slabs = {}
    for bi, (ts, L, widx) in enumerate(blocks):
        Lc = L + (L % 2)
        slab = np.zeros((P, KO, Lc), dtype=BF16)
        slab[:, :, :L] = a16[ts:ts + L].reshape(L, KO, P).transpose(2, 1, 0)
        a_slabs[f"a{bi}"] = slab

    b16 = np.asarray(b, dtype=np.float32).astype(BF16)        # [E, N, K]
    in_maps = []
    for j in range(NC):
        w = np.empty((EA, P, KO, NS), dtype=BF16)
        for ei, e in enumerate(experts):
            # b[e] is [N, K] row-major; out = a @ b[e].T needs W^T = [K, NS]
            # wt[ei, p, ko, n] = b[e, j*NS + n, ko*128 + p]
            sl = b16[e, j * NS:(j + 1) * NS, :]               # [NS, K]
            w[ei] = sl.reshape(NS, KO, P).transpose(2, 1, 0)
        m = {"wt": w}
        m.update(a_slabs)
        in_maps.append(m)

    nc = _build_program(T, K, NS, EA, blocks)

    import os
    trace = bool(int(os.environ.get("BASS_KERNEL_TRACE", "0")))
    res = run_bass_kernel_spmd(nc, in_maps, list(range(NC)), trace=trace)
    LAST_RESULT["exec_time_ns"] = res.exec_time_ns
    LAST_RESULT["results"] = res

    out_t = np.empty((N, T), dtype=np.float32)
    for j in range(NC):
        for bi, (ts, L, widx) in enumerate(blocks):
            ob = res.results[j][f"o{bi}"].reshape(P, NB, L)
            out_t[j * NS:(j + 1) * NS, ts:ts + L] = (
                ob.transpose(1, 0, 2).reshape(NS, L).astype(np.float32))
    return np.ascontiguousarray(out_t.T)


# revision 33
# speedup vs baseline: 1.0209x; 1.0209x over previous
"""Grouped GEMM (MoE routing) Trainium2 kernel.

Strategy: tensor-parallel shard of the output N dim across 8 NeuronCores.
Every core sees all T=8192 tokens and a 512-wide slice of every expert's
weights, so per-core work is identical regardless of segment sizes and a
single SPMD program (with the segment boundaries baked in as compile-time
constants) runs on all 8 cores.

Inputs are cast to bf16 on the host (rel-err ~3e-3, well inside the 2e-2
gate), halving per-core HBM read traffic so the DMA hides under the matmul
roofline (~437 us/core).  The output is written back as bf16 too and
upcast on the host.

Token blocks (<=512 tokens, segment-aligned) are the unit of work.  The
host lays `a` out as one CONTIGUOUS slab per block ([P, KO, Lc]) and the
output as one slab per block ([P, NB*L]), so every DMA descriptor run is
multi-KB: the SDMA round-robin between the a-queue and the weight-queue is
packet-granular, and sub-KB a-descriptors would get starved ~9:1 whenever
a 4MB weight tile streams in.

Per core:  out_t[n, t] = sum_k w_t[e(t), k, n] * a_t[k, t]
  - a{b} : [P, KO, Lc_b]  per-block a slab, k-partition-major (shared)
  - wt   : [EA, P, KO, NS] per-core weight slices, k-partition-major
  - o{b} : [P, NB*L_b] bf16 per-block output slab

Matmul mapping: stationary lhsT = w tile [k=128, n=128] bf16, moving rhs =
a tile [k=128, tok<=512] bf16, PSUM out [n=128, tok<=512] fp32, accumulated
over the 32 k-chunks.

DMA ring split: a-tiles ride the sync (SP) HWDGE ring; weight loads and
output writes ride the scalar (ACT) ring.  Each expert's weight tile is
loaded in 1MB ko-chunks spread token-weighted over the blocks of the two
PREVIOUS expert runs (the wpool buffer is provably free by then), so there
is no weight burst competing with a-tiles at an expert switch.  Block 0
uses graded ko batches (2,2,4,8,16) so the first matmul starts ~10us in;
the last block is shrunk to 128 tokens so the final drain is short.
"""

import numpy as np
import ml_dtypes

import concourse.bacc as bacc
import concourse.bass as bass
import concourse.mybir as mybir
import concourse.tile as tile
from concourse.bass_utils import run_bass_kernel_spmd

NC = 8          # NeuronCores
P = 128         # partitions
TB = 512        # max token block (moving free dim, one PSUM bank of fp32)
KOC = 16        # k-chunks per a-tile DMA batch

BF16 = ml_dtypes.bfloat16

LAST_RESULT = {}


def _token_blocks(seg_starts, seg_ends):
    """Split each segment into even pieces of <=512 tokens; carve a small
    tail piece off the last segment so the final drain is short."""
    blocks = []  # (tstart, tlen, run_idx)
    nseg = len(seg_starts)
    for widx, (s, t) in enumerate(zip(seg_starts, seg_ends)):
        ln = t - s
        tail = 128 if (widx == nseg - 1 and ln > 256) else 0
        mid = ln - tail
        pieces = []
        if mid > 0:
            npieces = max(1, -(-mid // TB))
            base, rem = divmod(mid, npieces)
            pieces += [base + (1 if i < rem else 0) for i in range(npieces)]
        if tail:
            pieces.append(tail)
        p = s
        for L in pieces:
            if L > 0:
                blocks.append((p, L, widx))
                p += L
    return blocks


def _w_chunks(run_idx, KO):
    """ko-chunk splits for one expert's weight tile: (start_ko, n_ko)."""
    if run_idx == 0 and KO == 32:
        return [(0, 1), (1, 1), (2, 2), (4, 4), (8, 8), (16, 8), (24, 8)]
    step = 8 if KO % 8 == 0 else KO
    return [(ko, min(step, KO - ko)) for ko in range(0, KO, step)]


def _kon_sched(bi, KO, koc_n):
    """ko batch sizes for one block's a-tile DMAs; graded for block 0 so
    the first matmul only waits on a small first transfer."""
    if bi == 0 and KO >= 16 and (KO - 16) % koc_n == 0:
        return [1, 1, 2, 4, 8] + [koc_n] * ((KO - 16) // koc_n)
    return [koc_n] * (KO // koc_n)


def _build_program(T, K, NS, EA, blocks):
    f32 = mybir.dt.float32
    bf16 = mybir.dt.bfloat16
    KO = K // P
    NB = NS // P
    koc_n = min(KOC, KO)

    nc = bacc.Bacc(None, target_bir_lowering=False)
    wt = nc.declare_dram_parameter("wt", [EA, P, KO, NS], bf16, isOutput=False)
    a_prm = []
    o_prm = []
    for bi, (ts, L, widx) in enumerate(blocks):
        Lc = L + (L % 2)
        a_prm.append(nc.declare_dram_parameter(
            f"a{bi}", [P, KO, Lc], bf16, isOutput=False))
        o_prm.append(nc.declare_dram_parameter(
            f"o{bi}", [P, NB * L], bf16, isOutput=True))

    run_of_block = [b[2] for b in blocks]
    nrun = EA

    # prefetch plan: run r's weight chunks are spread token-weighted across
    # the blocks of runs r-2 and r-1 (run r's wpool buffer was last read by
    # run r-3, so it is free throughout), avoiding any weight burst at an
    # expert switch.  Run 0's chunks are emitted before the first block.
    blocks_of_run = {}
    for bi, r in enumerate(run_of_block):
        blocks_of_run.setdefault(r, []).append(bi)
    emit_after = {bi: [] for bi in range(len(blocks))}   # bi -> [(run, chunk)]
    for r in range(1, nrun):
        hosts = (blocks_of_run[r - 2] if r >= 2 else []) + blocks_of_run[r - 1]
        host_tok = [blocks[bi][1] for bi in hosts]
        cum = np.cumsum(host_tok)
        total = cum[-1]
        chunks = _w_chunks(r, KO)
        for ci, ch in enumerate(chunks):
            frac = (ci + 0.5) / len(chunks)
            if r == 1:
                # run 1's chunks compete with the first blocks' a-tiles
                # inside the slow DMA spin-up window; push them into the
                # second half of run 0's token span.
                frac = 0.5 + 0.5 * frac
            pos = frac * total
            host_bi = hosts[int(np.searchsorted(cum, pos))]
            emit_after[host_bi].append((r, ch))

    with tile.TileContext(nc) as tc:
        with (
            tc.tile_pool(name="wpool", bufs=3) as wpool,
            tc.tile_pool(name="apool", bufs=5) as apool,
            tc.tile_pool(name="opool", bufs=3) as opool,
            tc.tile_pool(name="warm", bufs=1) as warmpool,
            tc.tile_pool(name="psum", bufs=8, space=bass.MemorySpace.PSUM) as psum_pool,
        ):
            wtiles = {}

            def emit_w_chunk(r, ch):
                if r not in wtiles:
                    wtiles[r] = wpool.tile([P, KO, NS], bf16, tag="w",
                                           name=f"w{r % 3}")
                ko0, kon = ch
                nc.scalar.dma_start(
                    out=wtiles[r][:, ko0:ko0 + kon, :],
                    in_=wt[r, :, ko0:ko0 + kon, :],
                )

            # PE warmup: a few dummy matmuls on a zeroed scratch tile while
            # the first real DMAs stream in, nudging the HAM clock gate
            # toward 8/8 before real matmuls start.  scr cols 256:272 are a
            # landing pad for ring-priming DMAs (disjoint from MM operands).
            scr = warmpool.tile([P, 272], bf16, tag="scr", name="warm_scr")
            nc.vector.memset(scr[:, :], 0)
            warm_ps = psum_pool.tile([P, 128], f32, tag="ps", name="warm_ps",
                                     padded_shape=[P, TB])
            # prime both HWDGE rings with tiny reads so their first-packet
            # pipeline fill happens before the real transfers queue up.
            nc.sync.dma_start(out=scr[:, 256:264], in_=wt[0, :, 0, 0:8])
            nc.scalar.dma_start(out=scr[:, 264:272], in_=wt[0, :, 0, 8:16])
            for _ in range(10):
                nc.tensor.matmul(warm_ps[:, :], scr[:, 0:128], scr[:, 128:256],
                                 start=True, stop=True)

            for ch in _w_chunks(0, KO):
                emit_w_chunk(0, ch)

            for bi, (ts, L, widx) in enumerate(blocks):
                r = run_of_block[bi]
                w_tile = wtiles[r]
                Lc = L + (L % 2)
                ptiles = [psum_pool.tile([P, Lc], f32, tag="ps", name=f"ps{nb}",
                                         padded_shape=[P, TB])
                          for nb in range(NB)]
                ko0 = 0
                for kon in _kon_sched(bi, KO, koc_n):
                    # flat [P, kon*Lc] so src AND dst are fully contiguous
                    a_tile = apool.tile([P, kon * Lc], bf16, tag="a",
                                        name="a_tile",
                                        padded_shape=[P, koc_n * TB])
                    nc.sync.dma_start(
                        out=a_tile[:, :],
                        in_=a_prm[bi][:, ko0:ko0 + kon, :],
                    )
                    for koi in range(kon):
                        ko = ko0 + koi
                        for nb in range(NB):
                            nc.tensor.matmul(
                                ptiles[nb][:, :],
                                w_tile[:, ko, nb * P:(nb + 1) * P],
                                a_tile[:, koi * Lc:(koi + 1) * Lc],
                                start=(ko == 0),
                                stop=(ko == KO - 1),
                            )
                    ko0 += kon
                    if bi == 0 and ko0 in (4, 8, 16):
                        # keep the PE clock-gate warm through block 0's
                        # data-drip waits with a few dummy matmuls.
                        for _ in range(4):
                            nc.tensor.matmul(warm_ps[:, :], scr[:, 0:128],
                                             scr[:, 128:256],
                                             start=True, stop=True)
                o_tile = opool.tile([P, NB * L], bf16, tag="o", name="o_tile",
                                    padded_shape=[P, NB * TB])
                for nb in range(NB):
                    nc.vector.tensor_copy(o_tile[:, nb * L:(nb + 1) * L],
                                          ptiles[nb][:, 0:L])
                nc.scalar.dma_start(out=o_prm[bi][:, :], in_=o_tile[:, :])
                for (rr, ch) in emit_after[bi]:
                    emit_w_chunk(rr, ch)
    nc.compile()
    return nc


def kernel(a, b, c, seg_indptr, weight_indices, batch_size, **_):
    T, K = a.shape
    E, N, K2 = b.shape
    assert K == K2
    NS = N // NC

    seg = np.asarray(seg_indptr).astype(np.int64)
    widx_arr = np.asarray(weight_indices).astype(np.int64)
    segs = [(int(seg[e]), int(seg[e + 1]), int(widx_arr[e]))
            for e in range(int(batch_size)) if seg[e + 1] > seg[e]]
    seg_starts = [s for s, _, _ in segs]
    seg_ends = [t for _, t, _ in segs]
    experts = [w for _, _, w in segs]
    EA = len(segs)
    blocks = _token_blocks(seg_starts, seg_ends)

    KO = K // P
    NB = NS // P
    a16 = np.asarray(a, dtype=np.float32).astype(BF16)        # [T, K]

    # per-block contiguous a slabs: a{b}[p, ko, t'] = a[ts + t', ko*128 + p]
    a_slabs = {}
    for bi, (ts, L, widx) in enumerate(blocks):
        Lc = L + (L % 2)
        slab = np.zeros((P, KO, Lc), dtype=BF16)
        slab[:, :, :L] = a16[ts:ts + L].reshape(L, KO, P).transpose(2, 1, 0)
        a_slabs[f"a{bi}"] = slab

    b16 = np.asarray(b, dtype=np.float32).astype(BF16)        # [E, N, K]
    in_maps = []
    for j in range(NC):
        w = np.empty((EA, P, KO, NS), dtype=BF16)
        for ei, e in enumerate(experts):
            # b[e] is [N, K] row-major; out = a @ b[e].T needs W^T = [K, NS]
            # wt[ei, p, ko, n] = b[e, j*NS + n, ko*128 + p]
            sl = b16[e, j * NS:(j + 1) * NS, :]               # [NS, K]
            w[ei] = sl.reshape(NS, KO, P).transpose(2, 1, 0)
        m = {"wt": w}
        m.update(a_slabs)
        in_maps.append(m)

    nc = _build_program(T, K, NS, EA, blocks)

    import os
    trace = bool(int(os.environ.get("BASS_KERNEL_TRACE", "0")))
    res = run_bass_kernel_spmd(nc, in_maps, list(range(NC)), trace=trace)
    LAST_RESULT["exec_time_ns"] = res.exec_time_ns
    LAST_RESULT["results"] = res

    out_t = np.empty((N, T), dtype=np.float32)
    for j in range(NC):
        for bi, (ts, L, widx) in enumerate(blocks):
            ob = res.results[j][f"o{bi}"].reshape(P, NB, L)
            out_t[j * NS:(j + 1) * NS, ts:ts + L] = (
                ob.transpose(1, 0, 2).reshape(NS, L).astype(np.float32))
    return np.ascontiguousarray(out_t.T)
